# revision 20
# baseline (speedup 1.0000x reference)
"""Fused graph Fokker-Planck ODE function kernel for Trainium2 (8 NeuronCores).

Sharding: data-parallel over batch B=4 x row-halves (i in [0,256) / [256,512))
-> 8 shards.  Each core computes dh_dt for one (batch, i-half) pair.

Math (per core; S/X/M kept transposed as [j, i] on chip so the j-contraction
matmul needs no transposes):
    S      = A^T * (Q K^T) / sqrt(D)          [j, i]  (elementwise mask)
    X      = exp(S)                            (unnormalized softmax; masked
                                                scores are O(5) so no max sub)
    rd     = 1/(1+exp(10(E_j-E_i))) = 1 - sigmoid(10(E_j-E_i))
             built separably: exp(10Ej)[j] (x) exp(-10Ei)[i] via a PE rank-1
    M4     = X * rd
    F_jd   = E_j + beta_d * ln(h_jd + 1e-8)   (fuses the E and beta*log terms)
    P      = [h | Fh | F | 1]^T @ [X | M4]    one accumulating matmul chain:
             stationary = value columns (padded to 128), moving = [X | M4]
             -> P[0:32]  = GXh | G4h      P[32:64] = GXFh | G4Fh
                P[64:96] = (GXF) | G4F    P[96]    = sX | r4
    c1     = E_i + beta_d * ln(h_id + 1e-8)   [d, i]
    dh^T   = ( (GXFh-G4Fh) - c1*(GXh-G4h) + h_i^T*(G4F - c1*r4) ) / sX
All finals run in the transposed [d, i] orientation (beta and bk/bq become
per-partition scalars); the host gather un-transposes.
"""

import math
import sys

import numpy as np

for _p in ("/opt/trn_rl_repo",):
    if _p not in sys.path:
        sys.path.insert(0, _p)

B, N, D, PED = 4, 512, 32, 16
NCORES = 8
RPC = N // 2            # i-rows per core
NJT = N // 128          # j tiles of 128
GW = 128                # stationary columns [h | Fh | F | 1 | pad]
KSH = 10.0
ISD = 1.0 / math.sqrt(D)

# aux1 column layout
A_EJ = 0                # [128, 4]   E_j per j-tile
A_BB = 4                # [128, 32]  beta broadcast
A_BK = 36               # [0:32, 1]  bk
A_BQ = 37               # [0:32, 1]  bq
A_B32 = 38              # [0:32, 1]  beta column
A_EIR = 40              # [0:1, 256] E_i row
A_W = 296

_CACHE = {}


def _patch_act_tables():
    """Make natural_log_exp_and_others the only ACT table set containing our
    functions (exp/ln/identity/copy) so bacc emits exactly one
    ACT_TABLE_LOAD.  Dict length/order is preserved — the set INDEX is the
    runtime act_func_set_id, so entries must not be removed."""
    import concourse.bacc as bacc_mod
    if getattr(bacc_mod, "_act_tables_patched", False):
        return
    orig = bacc_mod.get_activation_tables

    def filtered(arch):
        t = orig(arch)
        target = t.get("natural_log_exp_and_others")
        if not target:
            return t
        return {k: (v if k == "natural_log_exp_and_others" else (v - target))
                for k, v in t.items()}

    bacc_mod.get_activation_tables = filtered
    bacc_mod._act_tables_patched = True


def _build_program():
    import concourse.bacc as bacc
    import concourse.tile as tile
    from concourse import mybir
    from contextlib import ExitStack

    _patch_act_tables()

    fp32 = mybir.dt.float32
    f32r = mybir.dt.float32r
    AF = mybir.ActivationFunctionType
    ADD, MUL = mybir.AluOpType.add, mybir.AluOpType.mult
    SUB = mybir.AluOpType.subtract

    nc = bacc.Bacc("TRN2", target_bir_lowering=False, debug=False,
                   num_devices=NCORES)

    def din(name, shape):
        return nc.dram_tensor(name, shape, fp32, kind="ExternalInput").ap()

    AT = din("AT", [128, NJT * RPC])      # host-permuted [p, (t i)]
    hj = din("hj", [128, NJT * D])        # host-permuted [p, (t d)]
    pewkq = din("pewkq", [PED, N + RPC + 2 * D])   # [peT | peiT | Wk | Wq]
    aux1 = din("aux1", [128, A_W])
    aux2 = din("aux2", [D, 2 * RPC])      # [h_i^T | E_i broadcast]
    out = nc.dram_tensor("out", [D, RPC], fp32, kind="ExternalOutput").ap()

    with tile.TileContext(nc) as tc, ExitStack() as ctx:
        cst = ctx.enter_context(tc.tile_pool(name="cst", bufs=1))
        sb = ctx.enter_context(tc.tile_pool(name="sb", bufs=1))
        fin = ctx.enter_context(tc.tile_pool(name="fin", bufs=1))
        pq = ctx.enter_context(tc.tile_pool(name="pq", bufs=1, space="PSUM"))
        sps = ctx.enter_context(tc.tile_pool(name="sps", bufs=1, space="PSUM"))
        fps = ctx.enter_context(tc.tile_pool(name="fps", bufs=1, space="PSUM"))

        # --- input DMAs: all issued from the idle sync engine (a
        # DMA_DIRECT2D blocks its issuing engine ~650ns), ordered by need ---
        pewkq_sb = cst.tile([PED, N + RPC + 2 * D], fp32, tag="pewkq_sb")
        nc.sync.dma_start(pewkq_sb[:], pewkq[:])
        aux_sb = cst.tile([128, A_W], fp32, tag="aux_sb")
        nc.sync.dma_start(aux_sb[:], aux1[:])
        at_all = cst.tile([128, NJT * RPC], fp32, tag="at_all")
        HW0 = NJT * RPC // 2
        nc.sync.dma_start(at_all[:, 0:HW0], AT[:, 0:HW0])
        hj_sb = cst.tile([128, NJT * D], fp32, tag="hj_sb")
        nc.sync.dma_start(hj_sb[:], hj[:])
        nc.sync.dma_start(at_all[:, HW0:2 * HW0], AT[:, HW0:2 * HW0])
        aux2_sb = cst.tile([D, 2 * RPC], fp32, tag="aux2_sb")
        nc.sync.dma_start(aux2_sb[:], aux2[:])

        ej = aux_sb[:, A_EJ:A_EJ + NJT]
        betab = aux_sb[:, A_BB:A_BB + D]
        bk = aux_sb[0:D, A_BK:A_BK + 1]
        bq = aux_sb[0:D, A_BQ:A_BQ + 1]
        b32 = aux_sb[0:D, A_B32:A_B32 + 1]
        eirow = aux_sb[0:1, A_EIR:A_EIR + RPC]
        hiT = aux2_sb[:, 0:RPC]
        eibt = aux2_sb[:, RPC:2 * RPC]

        # ---------------- consts ------------------------------------------
        zero1 = cst.tile([128, 1], fp32, tag="zero1")
        nc.vector.memset(zero1[:], 0.0)
        eps1 = cst.tile([128, 1], fp32, tag="eps1")
        nc.vector.memset(eps1[:], 1e-8)
        ones128 = cst.tile([1, 128], f32r, tag="ones128")
        nc.vector.memset(ones128.bitcast(fp32)[:], 1.0)
        # dummy first ACT op: hoists the one ACT_TABLE_LOAD off the
        # critical path (it otherwise waits for the first real input)
        warm = cst.tile([128, 1], fp32, tag="warm")
        nc.scalar.activation(warm[:], zero1[:], AF.Exp, bias=zero1[:])

        # ------- sign split rd = 1/(1 + exp(10Ej)*exp(-10Ei)) --------------
        brow = cst.tile([1, RPC], f32r, tag="brow")
        nc.scalar.activation(brow[:], eirow, AF.Exp, bias=zero1[0:1, :],
                             scale=-KSH)
        acol = cst.tile([128, NJT], fp32, tag="acol")
        nc.scalar.activation(acol[:], ej, AF.Exp, bias=zero1[:], scale=KSH)
        bbps = pq.tile([128, 2 * RPC], fp32, tag="bbps")
        nc.tensor.matmul(bbps[:, 0:RPC], ones128[:], brow[:],
                         start=True, stop=True)
        d1 = sb.tile([128, NJT * RPC], fp32, tag="d1")
        rd = sb.tile([128, NJT * RPC], fp32, tag="rd")

        def mk_rd(jt):
            nc.vector.tensor_scalar(d1[:, jt * RPC:(jt + 1) * RPC],
                                    bbps[:, 0:RPC],
                                    acol[:, jt:jt + 1], 1.0,
                                    op0=MUL, op1=ADD)
            nc.vector.reciprocal_approx_fast(rd[:, jt * RPC:(jt + 1) * RPC],
                                             d1[:, jt * RPC:(jt + 1) * RPC])
        mk_rd(0)

        # ---------------- K^T, Q^T ----------------------------------------
        pewkq_r = cst.tile([PED, N + RPC + 2 * D], f32r, tag="pewkq_r")
        nc.vector.tensor_copy(pewkq_r[:, N:N + RPC + 2 * D],
                              pewkq_sb[:, N:N + RPC + 2 * D])
        nc.vector.tensor_copy(pewkq_r[:, 0:N], pewkq_sb[:, 0:N])
        peT = pewkq_r[:, 0:N]
        peiT = pewkq_r[:, N:N + RPC]
        wk = pewkq_r[:, N + RPC:N + RPC + D]
        wq = pewkq_r[:, N + RPC + D:N + RPC + 2 * D]
        kps = pq.tile([D, 2 * RPC], fp32, tag="kps")   # bank; use 0:RPC
        nc.tensor.matmul(kps[:, 0:RPC], wk, peiT, start=True, stop=True)
        kT = cst.tile([D, RPC], f32r, tag="kT")
        # (K + bk) * (1/sqrt(D))
        nc.vector.tensor_scalar(kT[:], kps[:, 0:RPC], bk, ISD,
                                op0=ADD, op1=MUL)
        qps = pq.tile([D, N], fp32, tag="qps")
        nc.tensor.matmul(qps[:], wq, peT, start=True, stop=True)
        qT = cst.tile([D, N], f32r, tag="qT")
        nc.scalar.activation(qT[:], qps[:], AF.Identity, bias=bq, scale=1.0)

        # ------------- rhs columns [h | Fh | F | 1 | pad] ------------------
        rhs_all = cst.tile([128, NJT * GW], f32r, tag="rhs_all")
        rv = rhs_all.rearrange("p (t c) -> p t c", c=GW)
        hv = hj_sb.rearrange("p (t d) -> p t d", d=D)
        Lt = sb.tile([128, NJT * D], fp32, tag="Lt")
        nc.scalar.activation(Lt[:], hj_sb[:], AF.Ln, bias=eps1[:])
        Lv = Lt.rearrange("p (t d) -> p t d", d=D)
        bbv = betab.rearrange("p (t d) -> p t d", t=1).to_broadcast(
            (128, NJT, D))
        Ft = sb.tile([128, NJT * D], fp32, tag="Ft")
        Fv = Ft.rearrange("p (t d) -> p t d", d=D)
        nc.gpsimd.tensor_mul(Fv[:], Lv[:], bbv)        # beta*ln(h)
        ejb = ej.rearrange("p (t o) -> p t o", o=1).to_broadcast((128, NJT, D))
        nc.gpsimd.tensor_tensor(rv[:, :, 2 * D:3 * D], Fv[:], ejb, op=ADD)
        nc.scalar.activation(rv[:, :, 0:D], hv[:], AF.Identity, bias=zero1[:])
        nc.gpsimd.tensor_mul(rv[:, :, D:2 * D], rv[:, :, 2 * D:3 * D], hv[:])
        # ones replicated on cols 96:128 -> P[96:128] = [sX | r4] broadcast
        nc.vector.memset(rv[:, :, 3 * D:GW].bitcast(fp32), 1.0)

        # ---------------- per-j-tile pipeline -----------------------------
        # sallA holds jt0|jt2, sallB jt1|jt3 so V-reads and PE-writes of
        # consecutive tiles land in different PSUM banks.
        sallA = sps.tile([128, 2 * RPC], fp32, tag="sallA")
        sallB = sps.tile([128, 2 * RPC], fp32, tag="sallB")
        P = fps.tile([GW, 2 * RPC], fp32, tag="P")
        XM = cst.tile([128, NJT * 2 * RPC], f32r, tag="XM")
        msk = sb.tile([128, NJT * RPC], fp32, tag="msk")
        for jt in range(NJT):
            bank = (sallA, sallB)[jt % 2]
            sl = slice((jt // 2) * RPC, (jt // 2 + 1) * RPC)
            nc.tensor.matmul(bank[:, sl], qT[:, jt * 128:(jt + 1) * 128],
                             kT[:], start=True, stop=True)
            nc.vector.tensor_mul(msk[:, jt * RPC:(jt + 1) * RPC],
                                 at_all[:, jt * RPC:(jt + 1) * RPC],
                                 bank[:, sl])
            x0 = jt * 2 * RPC
            nc.scalar.activation(XM[:, x0:x0 + RPC],
                                 msk[:, jt * RPC:(jt + 1) * RPC],
                                 AF.Exp, bias=zero1[:])
            m4eng = nc.vector if jt % 2 == 0 else nc.gpsimd
            m4eng.tensor_mul(XM[:, x0 + RPC:x0 + 2 * RPC],
                             XM[:, x0:x0 + RPC],
                             rd[:, jt * RPC:(jt + 1) * RPC])
            nc.tensor.matmul(P[:], rhs_all[:, jt * GW:(jt + 1) * GW],
                             XM[:, x0:x0 + 2 * RPC],
                             start=(jt == 0), stop=(jt == NJT - 1))
            if jt < NJT - 1:
                mk_rd(jt + 1)

        # ---------------- finals prep (transposed [d, i]) -----------------
        LiT = fin.tile([D, RPC], fp32, tag="LiT")
        nc.scalar.activation(LiT[:], hiT, AF.Ln, bias=eps1[0:D, :])
        cb = fin.tile([D, RPC], fp32, tag="cb")
        nc.vector.tensor_scalar_mul(cb[:], LiT[:], b32)
        c1 = fin.tile([D, RPC], fp32, tag="c1")
        nc.gpsimd.tensor_add(c1[:], cb[:], eibt)
        hc1 = fin.tile([D, RPC], fp32, tag="hc1")
        nc.gpsimd.tensor_mul(hc1[:], hiT, c1[:])

        # ---------------- finals ------------------------------------------
        # P rows: 0:32 GXh|G4h, 32:64 GXFh|G4Fh, 64:96 .|G4F,
        #         96:128 [sX | r4] already broadcast (replicated ones cols)
        r4b = P[3 * D:4 * D, RPC:2 * RPC]
        # TT can read only one PSUM operand: evacuate the GX half via ACT
        gx = fin.tile([2 * D, RPC], fp32, tag="gx")
        nc.scalar.activation(gx[:], P[0:2 * D, 0:RPC], AF.Identity,
                             bias=zero1[0:2 * D, :], scale=1.0)
        w1 = fin.tile([2 * D, RPC], fp32, tag="w1")   # [g3h; g3Fh]
        nc.vector.tensor_tensor(w1[:], gx[:], P[0:2 * D, RPC:2 * RPC],
                                op=SUB)
        # reciprocal_approx_fast mis-reads PSUM at partition base 96 on HW
        # (sim is fine) — evacuate sX rows via ACT, recip from SBUF
        sxs = fin.tile([D, RPC], fp32, tag="sxs")
        nc.scalar.activation(sxs[:], P[3 * D:4 * D, 0:RPC], AF.Identity,
                             bias=zero1[0:D, :], scale=1.0)
        invsb = fin.tile([D, RPC], fp32, tag="invsb")
        nc.vector.reciprocal_approx_fast(invsb[:], sxs[:])
        t3a = fin.tile([D, RPC], fp32, tag="t3a")
        nc.vector.tensor_mul(t3a[:], hiT, P[2 * D:3 * D, RPC:2 * RPC])
        t3b = fin.tile([D, RPC], fp32, tag="t3b")
        nc.vector.tensor_mul(t3b[:], hc1[:], r4b)
        # TT inputs must share a partition base: write t4 into rows 32:64
        # so the t5 subtraction sees both operands at base 32
        t4w = fin.tile([2 * D, RPC], fp32, tag="t4w")
        nc.gpsimd.tensor_mul(t4w[D:2 * D, :], c1[:], w1[0:D, :])
        t5 = fin.tile([D, RPC], fp32, tag="t5")
        nc.vector.tensor_tensor(t5[:], w1[D:2 * D, :], t4w[D:2 * D, :],
                                op=SUB)
        t6 = fin.tile([D, RPC], fp32, tag="t6")
        nc.vector.tensor_tensor(t6[:], t3a[:], t3b[:], op=SUB)
        numt = fin.tile([D, RPC], fp32, tag="numt")
        nc.vector.tensor_add(numt[:], t5[:], t6[:])
        res = fin.tile([D, RPC], fp32, tag="res")
        nc.vector.tensor_mul(res[:], numt[:], invsb[:])
        nc.sync.dma_start(out[:], res[:])

    nc.compile()
    return nc


def _get_program():
    if "nc" not in _CACHE:
        _CACHE["nc"] = _build_program()
    return _CACHE["nc"]


def make_in_maps(h, pe, E, A, Wk, bk, Wq, bq, beta):
    f = lambda x: np.ascontiguousarray(np.asarray(x, dtype=np.float32))
    h, pe, E, A = f(h), f(pe), f(E), f(A)
    Wk, bk, Wq, bq, beta = f(Wk), f(bk), f(Wq), f(bq), f(beta)
    in_maps = []
    for c in range(NCORES):
        b, r = c // 2, c % 2
        isl = slice(r * RPC, (r + 1) * RPC)
        atp = A[isl].T.reshape(NJT, 128, RPC).transpose(1, 0, 2)
        hjp = h[b].reshape(NJT, 128, D).transpose(1, 0, 2)
        pewkq = np.concatenate(
            [pe[b].T, pe[b, isl].T, Wk, Wq], axis=1)
        aux1 = np.zeros((128, A_W), np.float32)
        aux1[:, A_EJ:A_EJ + NJT] = E.reshape(NJT, 128).T
        aux1[:, A_BB:A_BB + D] = beta
        aux1[0:D, A_BK] = bk
        aux1[0:D, A_BQ] = bq
        aux1[0:D, A_B32] = beta
        aux1[0, A_EIR:A_EIR + RPC] = E[isl]
        aux2 = np.empty((D, 2 * RPC), np.float32)
        aux2[:, 0:RPC] = h[b, isl].T
        aux2[:, RPC:2 * RPC] = E[isl]
        in_maps.append({
            "AT": f(atp.reshape(128, NJT * RPC)),
            "hj": f(hjp.reshape(128, NJT * D)),
            "pewkq": f(pewkq),
            "aux1": aux1,
            "aux2": aux2,
        })
    return in_maps


def gather(results):
    out = np.empty((B, N, D), np.float32)
    for c in range(NCORES):
        b, r = c // 2, c % 2
        out[b, r * RPC:(r + 1) * RPC] = results[c]["out"].T
    return out


def _axon_reset():
    try:
        import ctypes
        import jax
        lib = ctypes.CDLL("/opt/axon/libaxon_pjrt.so")
        lib.axon_reset.restype = ctypes.c_int64
        jax.devices()
        lib.axon_reset()
    except Exception:
        pass


def kernel(t=None, h=None, pe=None, E=None, A=None, Wk=None, bk=None,
           Wq=None, bq=None, beta=None, **_unused):
    from concourse.bass_utils import run_bass_kernel_spmd
    nc = _get_program()
    in_maps = make_in_maps(h, pe, E, A, Wk, bk, Wq, bq, beta)
    try:
        res = run_bass_kernel_spmd(nc, in_maps, list(range(NCORES)))
    except Exception:
        # a previously wedged NeuronCore shows up as an opaque runtime
        # error on the first execute — reset the device once and retry
        _axon_reset()
        import time as _time
        _time.sleep(2)
        res = run_bass_kernel_spmd(nc, in_maps, list(range(NCORES)))
    return gather(res.results)


# revision 21
# speedup vs baseline: 1.0148x; 1.0148x over previous
"""Fused graph Fokker-Planck ODE function kernel for Trainium2 (8 NeuronCores).

Sharding: data-parallel over batch B=4 x row-halves (i in [0,256) / [256,512))
-> 8 shards.  Each core computes dh_dt for one (batch, i-half) pair.

Math (per core; S/X/M kept transposed as [j, i] on chip so the j-contraction
matmul needs no transposes):
    S      = A^T * (Q K^T) / sqrt(D)          [j, i]  (elementwise mask)
    X      = exp(S)                            (unnormalized softmax; masked
                                                scores are O(5) so no max sub)
    rd     = 1/(1+exp(10(E_j-E_i))) = 1 - sigmoid(10(E_j-E_i))
             built separably: exp(10Ej)[j] (x) exp(-10Ei)[i] via a PE rank-1
    M4     = X * rd
    F_jd   = E_j + beta_d * ln(h_jd + 1e-8)   (fuses the E and beta*log terms)
    P      = [h | Fh | F | 1]^T @ [X | M4]    one accumulating matmul chain:
             stationary = value columns (padded to 128), moving = [X | M4]
             -> P[0:32]  = GXh | G4h      P[32:64] = GXFh | G4Fh
                P[64:96] = (GXF) | G4F    P[96]    = sX | r4
    c1     = E_i + beta_d * ln(h_id + 1e-8)   [d, i]
    dh^T   = ( (GXFh-G4Fh) - c1*(GXh-G4h) + h_i^T*(G4F - c1*r4) ) / sX
All finals run in the transposed [d, i] orientation (beta and bk/bq become
per-partition scalars); the host gather un-transposes.
"""

import math
import sys

import numpy as np

for _p in ("/opt/trn_rl_repo",):
    if _p not in sys.path:
        sys.path.insert(0, _p)

B, N, D, PED = 4, 512, 32, 16
NCORES = 8
RPC = N // 2            # i-rows per core
NJT = N // 128          # j tiles of 128
GW = 128                # stationary columns [h | Fh | F | 1 | pad]
KSH = 10.0
ISD = 1.0 / math.sqrt(D)

# aux1 column layout
A_EJ = 0                # [128, 4]   E_j per j-tile
A_BB = 4                # [128, 32]  beta broadcast
A_BK = 36               # [0:32, 1]  bk
A_BQ = 37               # [0:32, 1]  bq
A_B32 = 38              # [0:32, 1]  beta column
A_EIR = 40              # [0:1, 256] E_i row
A_W = 296

_CACHE = {}


def _patch_act_tables():
    """Make natural_log_exp_and_others the only ACT table set containing our
    functions (exp/ln/identity/copy) so bacc emits exactly one
    ACT_TABLE_LOAD.  Dict length/order is preserved — the set INDEX is the
    runtime act_func_set_id, so entries must not be removed."""
    import concourse.bacc as bacc_mod
    if getattr(bacc_mod, "_act_tables_patched", False):
        return
    orig = bacc_mod.get_activation_tables

    def filtered(arch):
        t = orig(arch)
        target = t.get("natural_log_exp_and_others")
        if not target:
            return t
        return {k: (v if k == "natural_log_exp_and_others" else (v - target))
                for k, v in t.items()}

    bacc_mod.get_activation_tables = filtered
    bacc_mod._act_tables_patched = True


def _build_program():
    import concourse.bacc as bacc
    import concourse.tile as tile
    from concourse import mybir
    from contextlib import ExitStack

    _patch_act_tables()

    fp32 = mybir.dt.float32
    f32r = mybir.dt.float32r
    AF = mybir.ActivationFunctionType
    ADD, MUL = mybir.AluOpType.add, mybir.AluOpType.mult
    SUB = mybir.AluOpType.subtract

    nc = bacc.Bacc("TRN2", target_bir_lowering=False, debug=False,
                   num_devices=NCORES)

    def din(name, shape):
        return nc.dram_tensor(name, shape, fp32, kind="ExternalInput").ap()

    AT = din("AT", [128, NJT * RPC])      # host-permuted [p, (t i)]
    hj = din("hj", [128, NJT * D])        # host-permuted [p, (t d)]
    pewkq = din("pewkq", [PED, N + RPC + 2 * D])   # [peT | peiT | Wk | Wq]
    aux1 = din("aux1", [128, A_W])
    aux2 = din("aux2", [D, 2 * RPC])      # [h_i^T | E_i broadcast]
    out = nc.dram_tensor("out", [D, RPC], fp32, kind="ExternalOutput").ap()

    with tile.TileContext(nc) as tc, ExitStack() as ctx:
        cst = ctx.enter_context(tc.tile_pool(name="cst", bufs=1))
        sb = ctx.enter_context(tc.tile_pool(name="sb", bufs=1))
        fin = ctx.enter_context(tc.tile_pool(name="fin", bufs=1))
        pq = ctx.enter_context(tc.tile_pool(name="pq", bufs=1, space="PSUM"))
        sps = ctx.enter_context(tc.tile_pool(name="sps", bufs=1, space="PSUM"))
        fps = ctx.enter_context(tc.tile_pool(name="fps", bufs=1, space="PSUM"))

        # --- input DMAs: all issued from the idle sync engine (a
        # DMA_DIRECT2D blocks its issuing engine ~650ns), ordered by need ---
        pewkq_sb = cst.tile([PED, N + RPC + 2 * D], fp32, tag="pewkq_sb")
        nc.sync.dma_start(pewkq_sb[:], pewkq[:])
        aux_sb = cst.tile([128, A_W], fp32, tag="aux_sb")
        nc.sync.dma_start(aux_sb[:], aux1[:])
        at_all = cst.tile([128, NJT * RPC], fp32, tag="at_all")
        HW0 = NJT * RPC // 2
        nc.sync.dma_start(at_all[:, 0:HW0], AT[:, 0:HW0])
        hj_sb = cst.tile([128, NJT * D], fp32, tag="hj_sb")
        nc.sync.dma_start(hj_sb[:], hj[:])
        nc.sync.dma_start(at_all[:, HW0:2 * HW0], AT[:, HW0:2 * HW0])
        aux2_sb = cst.tile([D, 2 * RPC], fp32, tag="aux2_sb")
        nc.sync.dma_start(aux2_sb[:], aux2[:])

        ej = aux_sb[:, A_EJ:A_EJ + NJT]
        betab = aux_sb[:, A_BB:A_BB + D]
        bk = aux_sb[0:D, A_BK:A_BK + 1]
        bq = aux_sb[0:D, A_BQ:A_BQ + 1]
        b32 = aux_sb[0:D, A_B32:A_B32 + 1]
        eirow = aux_sb[0:1, A_EIR:A_EIR + RPC]
        hiT = aux2_sb[:, 0:RPC]
        eibt = aux2_sb[:, RPC:2 * RPC]

        # ---------------- consts ------------------------------------------
        zero1 = cst.tile([128, 1], fp32, tag="zero1")
        nc.vector.memset(zero1[:], 0.0)
        eps1 = cst.tile([128, 1], fp32, tag="eps1")
        nc.vector.memset(eps1[:], 1e-8)
        ones128 = cst.tile([1, 128], f32r, tag="ones128")
        nc.vector.memset(ones128.bitcast(fp32)[:], 1.0)
        # dummy first ACT op: hoists the one ACT_TABLE_LOAD off the
        # critical path (it otherwise waits for the first real input)
        warm = cst.tile([128, 1], fp32, tag="warm")
        nc.scalar.activation(warm[:], zero1[:], AF.Exp, bias=zero1[:])

        # ------- sign split rd = 1/(1 + exp(10Ej)*exp(-10Ei)) --------------
        brow = cst.tile([1, RPC], f32r, tag="brow")
        nc.scalar.activation(brow[:], eirow, AF.Exp, bias=zero1[0:1, :],
                             scale=-KSH)
        acol = cst.tile([128, NJT], fp32, tag="acol")
        nc.scalar.activation(acol[:], ej, AF.Exp, bias=zero1[:], scale=KSH)
        bbps = pq.tile([128, 2 * RPC], fp32, tag="bbps")
        nc.tensor.matmul(bbps[:, 0:RPC], ones128[:], brow[:],
                         start=True, stop=True)
        d1 = sb.tile([128, NJT * RPC], fp32, tag="d1")
        rd = sb.tile([128, NJT * RPC], fp32, tag="rd")

        def mk_rd(jt):
            nc.vector.tensor_scalar(d1[:, jt * RPC:(jt + 1) * RPC],
                                    bbps[:, 0:RPC],
                                    acol[:, jt:jt + 1], 1.0,
                                    op0=MUL, op1=ADD)
            nc.vector.reciprocal_approx_fast(rd[:, jt * RPC:(jt + 1) * RPC],
                                             d1[:, jt * RPC:(jt + 1) * RPC])
        mk_rd(0)

        # ---------------- K^T, Q^T ----------------------------------------
        pewkq_r = cst.tile([PED, N + RPC + 2 * D], f32r, tag="pewkq_r")
        nc.vector.tensor_copy(pewkq_r[:, N:N + RPC + 2 * D],
                              pewkq_sb[:, N:N + RPC + 2 * D])
        nc.vector.tensor_copy(pewkq_r[:, 0:N], pewkq_sb[:, 0:N])
        peT = pewkq_r[:, 0:N]
        peiT = pewkq_r[:, N:N + RPC]
        wk = pewkq_r[:, N + RPC:N + RPC + D]
        wq = pewkq_r[:, N + RPC + D:N + RPC + 2 * D]
        kps = pq.tile([D, 2 * RPC], fp32, tag="kps")   # bank; use 0:RPC
        nc.tensor.matmul(kps[:, 0:RPC], wk, peiT, start=True, stop=True)
        kT = cst.tile([D, RPC], f32r, tag="kT")
        # (K + bk) * (1/sqrt(D))
        nc.vector.tensor_scalar(kT[:], kps[:, 0:RPC], bk, ISD,
                                op0=ADD, op1=MUL)
        qps = pq.tile([D, N], fp32, tag="qps")
        nc.tensor.matmul(qps[:], wq, peT, start=True, stop=True)
        qT = cst.tile([D, N], f32r, tag="qT")
        nc.scalar.activation(qT[:], qps[:], AF.Identity, bias=bq, scale=1.0)

        # ------------- rhs columns [h | Fh | F | 1 | pad] ------------------
        rhs_all = cst.tile([128, NJT * GW], f32r, tag="rhs_all")
        rv = rhs_all.rearrange("p (t c) -> p t c", c=GW)
        hv = hj_sb.rearrange("p (t d) -> p t d", d=D)
        Lt = sb.tile([128, NJT * D], fp32, tag="Lt")
        nc.scalar.activation(Lt[:], hj_sb[:], AF.Ln, bias=eps1[:])
        Lv = Lt.rearrange("p (t d) -> p t d", d=D)
        bbv = betab.rearrange("p (t d) -> p t d", t=1).to_broadcast(
            (128, NJT, D))
        Ft = sb.tile([128, NJT * D], fp32, tag="Ft")
        Fv = Ft.rearrange("p (t d) -> p t d", d=D)
        nc.gpsimd.tensor_mul(Fv[:], Lv[:], bbv)        # beta*ln(h)
        ejb = ej.rearrange("p (t o) -> p t o", o=1).to_broadcast((128, NJT, D))
        nc.gpsimd.tensor_tensor(rv[:, :, 2 * D:3 * D], Fv[:], ejb, op=ADD)
        nc.scalar.activation(rv[:, :, 0:D], hv[:], AF.Identity, bias=zero1[:])
        nc.gpsimd.tensor_mul(rv[:, :, D:2 * D], rv[:, :, 2 * D:3 * D], hv[:])
        # ones replicated on cols 96:128 -> P[96:128] = [sX | r4] broadcast
        nc.vector.memset(rv[:, :, 3 * D:GW].bitcast(fp32), 1.0)

        # ---------------- per-j-tile pipeline -----------------------------
        # sallA holds jt0|jt2, sallB jt1|jt3 so V-reads and PE-writes of
        # consecutive tiles land in different PSUM banks.
        sallA = sps.tile([128, 2 * RPC], fp32, tag="sallA")
        sallB = sps.tile([128, 2 * RPC], fp32, tag="sallB")
        P = fps.tile([GW, 2 * RPC], fp32, tag="P")
        XM = cst.tile([128, NJT * 2 * RPC], f32r, tag="XM")
        msk = sb.tile([128, NJT * RPC], fp32, tag="msk")
        for jt in range(NJT):
            bank = (sallA, sallB)[jt % 2]
            sl = slice((jt // 2) * RPC, (jt // 2 + 1) * RPC)
            nc.tensor.matmul(bank[:, sl], qT[:, jt * 128:(jt + 1) * 128],
                             kT[:], start=True, stop=True)
            nc.vector.tensor_mul(msk[:, jt * RPC:(jt + 1) * RPC],
                                 at_all[:, jt * RPC:(jt + 1) * RPC],
                                 bank[:, sl])
            if jt < NJT - 1:
                mk_rd(jt + 1)
            x0 = jt * 2 * RPC
            nc.scalar.activation(XM[:, x0:x0 + RPC],
                                 msk[:, jt * RPC:(jt + 1) * RPC],
                                 AF.Exp, bias=zero1[:])
            m4eng = nc.vector if jt % 2 == 0 else nc.gpsimd
            m4eng.tensor_mul(XM[:, x0 + RPC:x0 + 2 * RPC],
                             XM[:, x0:x0 + RPC],
                             rd[:, jt * RPC:(jt + 1) * RPC])
            nc.tensor.matmul(P[:], rhs_all[:, jt * GW:(jt + 1) * GW],
                             XM[:, x0:x0 + 2 * RPC],
                             start=(jt == 0), stop=(jt == NJT - 1))

        # ---------------- finals prep (transposed [d, i]) -----------------
        LiT = fin.tile([D, RPC], fp32, tag="LiT")
        nc.scalar.activation(LiT[:], hiT, AF.Ln, bias=eps1[0:D, :])
        cb = fin.tile([D, RPC], fp32, tag="cb")
        nc.vector.tensor_scalar_mul(cb[:], LiT[:], b32)
        c1 = fin.tile([D, RPC], fp32, tag="c1")
        nc.vector.tensor_add(c1[:], cb[:], eibt)
        hc1 = fin.tile([D, RPC], fp32, tag="hc1")
        nc.vector.tensor_mul(hc1[:], hiT, c1[:])

        # ---------------- finals ------------------------------------------
        # P rows: 0:32 GXh|G4h, 32:64 GXFh|G4Fh, 64:96 .|G4F,
        #         96:128 [sX | r4] already broadcast (replicated ones cols)
        r4b = P[3 * D:4 * D, RPC:2 * RPC]
        # TT can read only one PSUM operand: evacuate the GX half via ACT
        gx = fin.tile([2 * D, RPC], fp32, tag="gx")
        nc.scalar.activation(gx[:], P[0:2 * D, 0:RPC], AF.Identity,
                             bias=zero1[0:2 * D, :], scale=1.0)
        w1 = fin.tile([2 * D, RPC], fp32, tag="w1")   # [g3h; g3Fh]
        nc.vector.tensor_tensor(w1[:], gx[:], P[0:2 * D, RPC:2 * RPC],
                                op=SUB)
        # reciprocal_approx_fast mis-reads PSUM at partition base 96 on HW
        # (sim is fine) — evacuate sX rows via ACT, recip from SBUF
        sxs = fin.tile([D, RPC], fp32, tag="sxs")
        nc.scalar.activation(sxs[:], P[3 * D:4 * D, 0:RPC], AF.Identity,
                             bias=zero1[0:D, :], scale=1.0)
        # TT inputs must share a partition base: write t4 into rows 32:64
        # so the t5 subtraction sees both operands at base 32
        t4w = fin.tile([2 * D, RPC], fp32, tag="t4w")
        nc.gpsimd.tensor_mul(t4w[D:2 * D, :], c1[:], w1[0:D, :])
        t3a = fin.tile([D, RPC], fp32, tag="t3a")
        nc.vector.tensor_mul(t3a[:], hiT, P[2 * D:3 * D, RPC:2 * RPC])
        t3b = fin.tile([D, RPC], fp32, tag="t3b")
        nc.vector.tensor_mul(t3b[:], hc1[:], r4b)
        t5 = fin.tile([D, RPC], fp32, tag="t5")
        nc.vector.tensor_tensor(t5[:], w1[D:2 * D, :], t4w[D:2 * D, :],
                                op=SUB)
        t6 = fin.tile([D, RPC], fp32, tag="t6")
        nc.vector.tensor_tensor(t6[:], t3a[:], t3b[:], op=SUB)
        numt = fin.tile([D, RPC], fp32, tag="numt")
        nc.vector.tensor_add(numt[:], t5[:], t6[:])
        invsb = fin.tile([D, RPC], fp32, tag="invsb")
        nc.vector.reciprocal_approx_fast(invsb[:], sxs[:])
        res = fin.tile([D, RPC], fp32, tag="res")
        nc.vector.tensor_mul(res[:], numt[:], invsb[:])
        nc.sync.dma_start(out[:], res[:])

    nc.compile()
    return nc


def _get_program():
    if "nc" not in _CACHE:
        _CACHE["nc"] = _build_program()
    return _CACHE["nc"]


def make_in_maps(h, pe, E, A, Wk, bk, Wq, bq, beta):
    f = lambda x: np.ascontiguousarray(np.asarray(x, dtype=np.float32))
    h, pe, E, A = f(h), f(pe), f(E), f(A)
    Wk, bk, Wq, bq, beta = f(Wk), f(bk), f(Wq), f(bq), f(beta)
    in_maps = []
    for c in range(NCORES):
        b, r = c // 2, c % 2
        isl = slice(r * RPC, (r + 1) * RPC)
        atp = A[isl].T.reshape(NJT, 128, RPC).transpose(1, 0, 2)
        hjp = h[b].reshape(NJT, 128, D).transpose(1, 0, 2)
        pewkq = np.concatenate(
            [pe[b].T, pe[b, isl].T, Wk, Wq], axis=1)
        aux1 = np.zeros((128, A_W), np.float32)
        aux1[:, A_EJ:A_EJ + NJT] = E.reshape(NJT, 128).T
        aux1[:, A_BB:A_BB + D] = beta
        aux1[0:D, A_BK] = bk
        aux1[0:D, A_BQ] = bq
        aux1[0:D, A_B32] = beta
        aux1[0, A_EIR:A_EIR + RPC] = E[isl]
        aux2 = np.empty((D, 2 * RPC), np.float32)
        aux2[:, 0:RPC] = h[b, isl].T
        aux2[:, RPC:2 * RPC] = E[isl]
        in_maps.append({
            "AT": f(atp.reshape(128, NJT * RPC)),
            "hj": f(hjp.reshape(128, NJT * D)),
            "pewkq": f(pewkq),
            "aux1": aux1,
            "aux2": aux2,
        })
    return in_maps


def gather(results):
    out = np.empty((B, N, D), np.float32)
    for c in range(NCORES):
        b, r = c // 2, c % 2
        out[b, r * RPC:(r + 1) * RPC] = results[c]["out"].T
    return out


def _axon_reset():
    try:
        import ctypes
        import jax
        lib = ctypes.CDLL("/opt/axon/libaxon_pjrt.so")
        lib.axon_reset.restype = ctypes.c_int64
        jax.devices()
        lib.axon_reset()
    except Exception:
        pass


def kernel(t=None, h=None, pe=None, E=None, A=None, Wk=None, bk=None,
           Wq=None, bq=None, beta=None, **_unused):
    from concourse.bass_utils import run_bass_kernel_spmd
    nc = _get_program()
    in_maps = make_in_maps(h, pe, E, A, Wk, bk, Wq, bq, beta)
    try:
        res = run_bass_kernel_spmd(nc, in_maps, list(range(NCORES)))
    except Exception:
        # a previously wedged NeuronCore shows up as an opaque runtime
        # error on the first execute — reset the device once and retry
        _axon_reset()
        import time as _time
        _time.sleep(2)
        res = run_bass_kernel_spmd(nc, in_maps, list(range(NCORES)))
    return gather(res.results)


# revision 22
# speedup vs baseline: 1.0332x; 1.0181x over previous
"""Fused graph Fokker-Planck ODE function kernel for Trainium2 (8 NeuronCores).

Sharding: data-parallel over batch B=4 x row-halves (i in [0,256) / [256,512))
-> 8 shards.  Each core computes dh_dt for one (batch, i-half) pair.

Math (per core; S/X/M kept transposed as [j, i] on chip so the j-contraction
matmul needs no transposes):
    S      = A^T * (Q K^T) / sqrt(D)          [j, i]  (elementwise mask)
    X      = exp(S)                            (unnormalized softmax; masked
                                                scores are O(5) so no max sub)
    rd     = 1/(1+exp(10(E_j-E_i))) = 1 - sigmoid(10(E_j-E_i))
             built separably: exp(10Ej)[j] (x) exp(-10Ei)[i] via a PE rank-1
    M4     = X * rd
    F_jd   = E_j + beta_d * ln(h_jd + 1e-8)   (fuses the E and beta*log terms)
    P      = [h | Fh | F | 1]^T @ [X | M4]    one accumulating matmul chain:
             stationary = value columns (padded to 128), moving = [X | M4]
             -> P[0:32]  = GXh | G4h      P[32:64] = GXFh | G4Fh
                P[64:96] = (GXF) | G4F    P[96]    = sX | r4
    c1     = E_i + beta_d * ln(h_id + 1e-8)   [d, i]
    dh^T   = ( (GXFh-G4Fh) - c1*(GXh-G4h) + h_i^T*(G4F - c1*r4) ) / sX
All finals run in the transposed [d, i] orientation (beta and bk/bq become
per-partition scalars); the host gather un-transposes.
"""

import math
import sys

import numpy as np

for _p in ("/opt/trn_rl_repo",):
    if _p not in sys.path:
        sys.path.insert(0, _p)

B, N, D, PED = 4, 512, 32, 16
NCORES = 8
RPC = N // 2            # i-rows per core
NJT = N // 128          # j tiles of 128
GW = 128                # stationary columns [h | Fh | F | 1 | pad]
KSH = 10.0
ISD = 1.0 / math.sqrt(D)

# aux1 column layout
A_EJ = 0                # [128, 4]   E_j per j-tile
A_BB = 4                # [128, 32]  beta broadcast
A_BK = 36               # [0:32, 1]  bk
A_BQ = 37               # [0:32, 1]  bq
A_B32 = 38              # [0:32, 1]  beta column
A_EIR = 40              # [0:1, 256] E_i row
A_PR = 296              # [128, 32]  +-1 pair-reduce stationary
A_W = 328

_CACHE = {}


def _patch_act_tables():
    """Make natural_log_exp_and_others the only ACT table set containing our
    functions (exp/ln/identity/copy) so bacc emits exactly one
    ACT_TABLE_LOAD.  Dict length/order is preserved — the set INDEX is the
    runtime act_func_set_id, so entries must not be removed."""
    import concourse.bacc as bacc_mod
    if getattr(bacc_mod, "_act_tables_patched", False):
        return
    orig = bacc_mod.get_activation_tables

    def filtered(arch):
        t = orig(arch)
        target = t.get("natural_log_exp_and_others")
        if not target:
            return t
        return {k: (v if k == "natural_log_exp_and_others" else (v - target))
                for k, v in t.items()}

    bacc_mod.get_activation_tables = filtered
    bacc_mod._act_tables_patched = True


def _build_program():
    import concourse.bacc as bacc
    import concourse.tile as tile
    from concourse import mybir
    from contextlib import ExitStack

    _patch_act_tables()

    fp32 = mybir.dt.float32
    f32r = mybir.dt.float32r
    AF = mybir.ActivationFunctionType
    ADD, MUL = mybir.AluOpType.add, mybir.AluOpType.mult
    SUB = mybir.AluOpType.subtract

    nc = bacc.Bacc("TRN2", target_bir_lowering=False, debug=False,
                   num_devices=NCORES)

    def din(name, shape):
        return nc.dram_tensor(name, shape, fp32, kind="ExternalInput").ap()

    AT = din("AT", [128, NJT * RPC])      # host-permuted [p, (t i)]
    hj = din("hj", [128, NJT * D])        # host-permuted [p, (t d)]
    pewkq = din("pewkq", [PED, N + RPC + 2 * D])   # [peT | peiT | Wk | Wq]
    aux1 = din("aux1", [128, A_W])
    aux2 = din("aux2", [D, 2 * RPC])      # [h_i^T | E_i broadcast]
    out = nc.dram_tensor("out", [D, RPC], fp32, kind="ExternalOutput").ap()

    with tile.TileContext(nc) as tc, ExitStack() as ctx:
        cst = ctx.enter_context(tc.tile_pool(name="cst", bufs=1))
        sb = ctx.enter_context(tc.tile_pool(name="sb", bufs=1))
        fin = ctx.enter_context(tc.tile_pool(name="fin", bufs=1))
        pq = ctx.enter_context(tc.tile_pool(name="pq", bufs=1, space="PSUM"))
        sps = ctx.enter_context(tc.tile_pool(name="sps", bufs=1, space="PSUM"))
        fps = ctx.enter_context(tc.tile_pool(name="fps", bufs=1, space="PSUM"))

        # --- input DMAs: all issued from the idle sync engine (a
        # DMA_DIRECT2D blocks its issuing engine ~650ns), ordered by need ---
        pewkq_sb = cst.tile([PED, N + RPC + 2 * D], fp32, tag="pewkq_sb")
        nc.sync.dma_start(pewkq_sb[:], pewkq[:])
        aux_sb = cst.tile([128, A_W], fp32, tag="aux_sb")
        nc.sync.dma_start(aux_sb[:], aux1[:])
        at_all = cst.tile([128, NJT * RPC], fp32, tag="at_all")
        HW0 = NJT * RPC // 2
        nc.sync.dma_start(at_all[:, 0:HW0], AT[:, 0:HW0])
        hj_sb = cst.tile([128, NJT * D], fp32, tag="hj_sb")
        nc.sync.dma_start(hj_sb[:], hj[:])
        nc.sync.dma_start(at_all[:, HW0:2 * HW0], AT[:, HW0:2 * HW0])
        aux2_sb = cst.tile([D, 2 * RPC], fp32, tag="aux2_sb")
        nc.sync.dma_start(aux2_sb[:], aux2[:])

        ej = aux_sb[:, A_EJ:A_EJ + NJT]
        betab = aux_sb[:, A_BB:A_BB + D]
        bk = aux_sb[0:D, A_BK:A_BK + 1]
        bq = aux_sb[0:D, A_BQ:A_BQ + 1]
        b32 = aux_sb[0:D, A_B32:A_B32 + 1]
        eirow = aux_sb[0:1, A_EIR:A_EIR + RPC]
        hiT = aux2_sb[:, 0:RPC]
        eibt = aux2_sb[:, RPC:2 * RPC]

        # ---------------- consts ------------------------------------------
        zero1 = cst.tile([128, 1], fp32, tag="zero1")
        nc.vector.memset(zero1[:], 0.0)
        eps1 = cst.tile([128, 1], fp32, tag="eps1")
        nc.vector.memset(eps1[:], 1e-8)
        ones128 = cst.tile([1, 128], f32r, tag="ones128")
        nc.vector.memset(ones128.bitcast(fp32)[:], 1.0)
        # dummy first ACT op: hoists the one ACT_TABLE_LOAD off the
        # critical path (it otherwise waits for the first real input)
        warm = cst.tile([128, 1], fp32, tag="warm")
        nc.scalar.activation(warm[:], zero1[:], AF.Exp, bias=zero1[:])

        # ------- sign split rd = 1/(1 + exp(10Ej)*exp(-10Ei)) --------------
        brow = cst.tile([1, RPC], f32r, tag="brow")
        nc.scalar.activation(brow[:], eirow, AF.Exp, bias=zero1[0:1, :],
                             scale=-KSH)
        acol = cst.tile([128, NJT], fp32, tag="acol")
        nc.scalar.activation(acol[:], ej, AF.Exp, bias=zero1[:], scale=KSH)
        bbps = pq.tile([128, 2 * RPC], fp32, tag="bbps")
        nc.tensor.matmul(bbps[:, 0:RPC], ones128[:], brow[:],
                         start=True, stop=True)
        d1 = sb.tile([128, NJT * RPC], fp32, tag="d1")
        rd = sb.tile([128, NJT * RPC], fp32, tag="rd")

        def mk_rd(jt):
            nc.vector.tensor_scalar(d1[:, jt * RPC:(jt + 1) * RPC],
                                    bbps[:, 0:RPC],
                                    acol[:, jt:jt + 1], 1.0,
                                    op0=MUL, op1=ADD)
            nc.vector.reciprocal_approx_fast(rd[:, jt * RPC:(jt + 1) * RPC],
                                             d1[:, jt * RPC:(jt + 1) * RPC])
        mk_rd(0)

        # ---------------- K^T, Q^T ----------------------------------------
        pewkq_r = cst.tile([PED, N + RPC + 2 * D], f32r, tag="pewkq_r")
        nc.vector.tensor_copy(pewkq_r[:, N:N + RPC + 2 * D],
                              pewkq_sb[:, N:N + RPC + 2 * D])
        nc.vector.tensor_copy(pewkq_r[:, 0:N], pewkq_sb[:, 0:N])
        peT = pewkq_r[:, 0:N]
        peiT = pewkq_r[:, N:N + RPC]
        wk = pewkq_r[:, N + RPC:N + RPC + D]
        wq = pewkq_r[:, N + RPC + D:N + RPC + 2 * D]
        kps = pq.tile([D, 2 * RPC], fp32, tag="kps")   # bank; use 0:RPC
        nc.tensor.matmul(kps[:, 0:RPC], wk, peiT, start=True, stop=True)
        kT = cst.tile([D, RPC], f32r, tag="kT")
        # (K + bk) * (1/sqrt(D))
        nc.vector.tensor_scalar(kT[:], kps[:, 0:RPC], bk, ISD,
                                op0=ADD, op1=MUL)
        qps = pq.tile([D, N], fp32, tag="qps")
        nc.tensor.matmul(qps[:], wq, peT, start=True, stop=True)
        qT = cst.tile([D, N], f32r, tag="qT")
        nc.scalar.activation(qT[:], qps[:], AF.Identity, bias=bq, scale=1.0)

        # ------------- rhs columns [h | Fh | F | 1 | pad] ------------------
        rhs_all = cst.tile([128, NJT * GW], f32r, tag="rhs_all")
        rv = rhs_all.rearrange("p (t c) -> p t c", c=GW)
        hv = hj_sb.rearrange("p (t d) -> p t d", d=D)
        Lt = sb.tile([128, NJT * D], fp32, tag="Lt")
        nc.scalar.activation(Lt[:], hj_sb[:], AF.Ln, bias=eps1[:])
        Lv = Lt.rearrange("p (t d) -> p t d", d=D)
        bbv = betab.rearrange("p (t d) -> p t d", t=1).to_broadcast(
            (128, NJT, D))
        Ft = sb.tile([128, NJT * D], fp32, tag="Ft")
        Fv = Ft.rearrange("p (t d) -> p t d", d=D)
        nc.gpsimd.tensor_mul(Fv[:], Lv[:], bbv)        # beta*ln(h)
        ejb = ej.rearrange("p (t o) -> p t o", o=1).to_broadcast((128, NJT, D))
        nc.gpsimd.tensor_tensor(rv[:, :, 2 * D:3 * D], Fv[:], ejb, op=ADD)
        nc.scalar.activation(rv[:, :, 0:D], hv[:], AF.Identity, bias=zero1[:])
        nc.gpsimd.tensor_mul(rv[:, :, D:2 * D], rv[:, :, 2 * D:3 * D], hv[:])
        # ones replicated on cols 96:128 -> P[96:128] = [sX | r4] broadcast
        nc.vector.memset(rv[:, :, 3 * D:GW].bitcast(fp32), 1.0)

        # ---------------- per-j-tile pipeline -----------------------------
        # sallA holds jt0|jt2, sallB jt1|jt3 so V-reads and PE-writes of
        # consecutive tiles land in different PSUM banks.
        sallA = sps.tile([128, 2 * RPC], fp32, tag="sallA")
        sallB = sps.tile([128, 2 * RPC], fp32, tag="sallB")
        P = fps.tile([GW, 2 * RPC], fp32, tag="P")
        XM = cst.tile([128, NJT * 2 * RPC], f32r, tag="XM")
        msk = sb.tile([128, NJT * RPC], fp32, tag="msk")
        for jt in range(NJT):
            bank = (sallA, sallB)[jt % 2]
            sl = slice((jt // 2) * RPC, (jt // 2 + 1) * RPC)
            nc.tensor.matmul(bank[:, sl], qT[:, jt * 128:(jt + 1) * 128],
                             kT[:], start=True, stop=True)
            nc.vector.tensor_mul(msk[:, jt * RPC:(jt + 1) * RPC],
                                 at_all[:, jt * RPC:(jt + 1) * RPC],
                                 bank[:, sl])
            if jt < NJT - 1:
                mk_rd(jt + 1)
            x0 = jt * 2 * RPC
            nc.scalar.activation(XM[:, x0:x0 + RPC],
                                 msk[:, jt * RPC:(jt + 1) * RPC],
                                 AF.Exp, bias=zero1[:])
            m4eng = nc.vector if jt % 2 == 0 else nc.gpsimd
            m4eng.tensor_mul(XM[:, x0 + RPC:x0 + 2 * RPC],
                             XM[:, x0:x0 + RPC],
                             rd[:, jt * RPC:(jt + 1) * RPC])
            nc.tensor.matmul(P[:], rhs_all[:, jt * GW:(jt + 1) * GW],
                             XM[:, x0:x0 + 2 * RPC],
                             start=(jt == 0), stop=(jt == NJT - 1))

        # ---------------- finals prep (transposed [d, i]) -----------------
        LiT = fin.tile([D, RPC], fp32, tag="LiT")
        nc.scalar.activation(LiT[:], hiT, AF.Ln, bias=eps1[0:D, :])
        cb = fin.tile([D, RPC], fp32, tag="cb")
        nc.vector.tensor_scalar_mul(cb[:], LiT[:], b32)
        c1 = fin.tile([D, RPC], fp32, tag="c1")
        nc.vector.tensor_add(c1[:], cb[:], eibt)
        # hh = [hiT; hiT*c1] feeds the fused [t3a; t3b] product
        hh = fin.tile([2 * D, RPC], fp32, tag="hh")
        nc.vector.tensor_copy(hh[0:D, :], hiT)
        nc.vector.tensor_mul(hh[D:2 * D, :], hiT, c1[:])
        pairs_r = cst.tile([128, D], f32r, tag="pairs_r")
        nc.vector.tensor_copy(pairs_r[:], aux_sb[:, A_PR:A_PR + D])

        # ---------------- finals ------------------------------------------
        # P rows: 0:32 GXh|G4h, 32:64 GXFh|G4Fh, 64:96 .|G4F,
        #         96:128 [sX | r4] already broadcast (replicated ones cols)
        # numt = g3Fh - c1*g3h + hiT*G4F - hiT*c1*r4 is reduced by ONE
        # pair-sum matmul over M = [t4 | g3Fh | t3a | t3b] (one 32-row
        # group each) with the +-1 stationary pairs_r.
        gx = fin.tile([2 * D, RPC], fp32, tag="gx")
        nc.scalar.activation(gx[:], P[0:2 * D, 0:RPC], AF.Identity,
                             bias=zero1[0:2 * D, :], scale=1.0)
        # reciprocal_approx_fast mis-reads PSUM at partition base 96 on HW
        # (sim is fine) — evacuate sX rows via ACT, recip from SBUF
        sxs = fin.tile([D, RPC], fp32, tag="sxs")
        nc.scalar.activation(sxs[:], P[3 * D:4 * D, 0:RPC], AF.Identity,
                             bias=zero1[0:D, :], scale=1.0)
        gh = fin.tile([D, RPC], fp32, tag="gh")       # g3h
        nc.vector.tensor_tensor(gh[:], gx[0:D, :], P[0:D, RPC:2 * RPC],
                                op=SUB)
        M = fin.tile([128, RPC], f32r, tag="M")
        nc.vector.tensor_tensor(M[D:2 * D, :], gx[D:2 * D, :],
                                P[D:2 * D, RPC:2 * RPC], op=SUB)  # g3Fh
        nc.vector.tensor_mul(M[2 * D:4 * D, :], hh[:],
                             P[2 * D:4 * D, RPC:2 * RPC])  # [t3a; t3b]
        nc.gpsimd.tensor_mul(M[0:D, :], c1[:], gh[:])          # t4
        nps = pq.tile([D, 2 * RPC], fp32, tag="nps")
        nc.tensor.matmul(nps[:, 0:RPC], pairs_r[:], M[:],
                         start=True, stop=True)
        invsb = fin.tile([D, RPC], fp32, tag="invsb")
        nc.vector.reciprocal_approx_fast(invsb[:], sxs[:])
        res = fin.tile([D, RPC], fp32, tag="res")
        nc.vector.tensor_mul(res[:], nps[:, 0:RPC], invsb[:])
        nc.sync.dma_start(out[:], res[:])

    nc.compile()
    return nc


def _get_program():
    if "nc" not in _CACHE:
        _CACHE["nc"] = _build_program()
    return _CACHE["nc"]


def make_in_maps(h, pe, E, A, Wk, bk, Wq, bq, beta):
    f = lambda x: np.ascontiguousarray(np.asarray(x, dtype=np.float32))
    h, pe, E, A = f(h), f(pe), f(E), f(A)
    Wk, bk, Wq, bq, beta = f(Wk), f(bk), f(Wq), f(bq), f(beta)
    in_maps = []
    for c in range(NCORES):
        b, r = c // 2, c % 2
        isl = slice(r * RPC, (r + 1) * RPC)
        atp = A[isl].T.reshape(NJT, 128, RPC).transpose(1, 0, 2)
        hjp = h[b].reshape(NJT, 128, D).transpose(1, 0, 2)
        pewkq = np.concatenate(
            [pe[b].T, pe[b, isl].T, Wk, Wq], axis=1)
        aux1 = np.zeros((128, A_W), np.float32)
        aux1[:, A_EJ:A_EJ + NJT] = E.reshape(NJT, 128).T
        aux1[:, A_BB:A_BB + D] = beta
        aux1[0:D, A_BK] = bk
        aux1[0:D, A_BQ] = bq
        aux1[0:D, A_B32] = beta
        aux1[0, A_EIR:A_EIR + RPC] = E[isl]
        pr = np.zeros((128, D), np.float32)
        idx = np.arange(D)
        pr[idx, idx] = -1.0          # -t4
        pr[D + idx, idx] = 1.0       # +g3Fh
        pr[2 * D + idx, idx] = 1.0   # +t3a
        pr[3 * D + idx, idx] = -1.0  # -t3b
        aux1[:, A_PR:A_PR + D] = pr
        aux2 = np.empty((D, 2 * RPC), np.float32)
        aux2[:, 0:RPC] = h[b, isl].T
        aux2[:, RPC:2 * RPC] = E[isl]
        in_maps.append({
            "AT": f(atp.reshape(128, NJT * RPC)),
            "hj": f(hjp.reshape(128, NJT * D)),
            "pewkq": f(pewkq),
            "aux1": aux1,
            "aux2": aux2,
        })
    return in_maps


def gather(results):
    out = np.empty((B, N, D), np.float32)
    for c in range(NCORES):
        b, r = c // 2, c % 2
        out[b, r * RPC:(r + 1) * RPC] = results[c]["out"].T
    return out


def _axon_reset():
    try:
        import ctypes
        import jax
        lib = ctypes.CDLL("/opt/axon/libaxon_pjrt.so")
        lib.axon_reset.restype = ctypes.c_int64
        jax.devices()
        lib.axon_reset()
    except Exception:
        pass


def kernel(t=None, h=None, pe=None, E=None, A=None, Wk=None, bk=None,
           Wq=None, bq=None, beta=None, **_unused):
    from concourse.bass_utils import run_bass_kernel_spmd
    nc = _get_program()
    in_maps = make_in_maps(h, pe, E, A, Wk, bk, Wq, bq, beta)
    try:
        res = run_bass_kernel_spmd(nc, in_maps, list(range(NCORES)))
    except Exception:
        # a previously wedged NeuronCore shows up as an opaque runtime
        # error on the first execute — reset the device once and retry
        _axon_reset()
        import time as _time
        _time.sleep(2)
        res = run_bass_kernel_spmd(nc, in_maps, list(range(NCORES)))
    return gather(res.results)


# revision 25
# speedup vs baseline: 1.0574x; 1.0234x over previous
"""Fused graph Fokker-Planck ODE function kernel for Trainium2 (8 NeuronCores).

Sharding: data-parallel over batch B=4 x row-halves (i in [0,256) / [256,512))
-> 8 shards.  Each core computes dh_dt for one (batch, i-half) pair.

Math (per core; S/X/M kept transposed as [j, i] on chip so the j-contraction
matmul needs no transposes):
    S      = A^T * (Q K^T) / sqrt(D)          [j, i]  (elementwise mask)
    X      = exp(S)                            (unnormalized softmax; masked
                                                scores are O(5) so no max sub)
    rd     = 1/(1+exp(10(E_j-E_i))) = 1 - sigmoid(10(E_j-E_i))
             built separably: exp(10Ej)[j] (x) exp(-10Ei)[i] via a PE rank-1
    M4     = X * rd
    F_jd   = E_j + beta_d * ln(h_jd + 1e-8)   (fuses the E and beta*log terms)
    P      = [h | Fh | F | 1]^T @ [X | M4]    one accumulating matmul chain:
             stationary = value columns (padded to 128), moving = [X | M4]
             -> P[0:32]  = GXh | G4h      P[32:64] = GXFh | G4Fh
                P[64:96] = (GXF) | G4F    P[96]    = sX | r4
    c1     = E_i + beta_d * ln(h_id + 1e-8)   [d, i]
    dh^T   = ( (GXFh-G4Fh) - c1*(GXh-G4h) + h_i^T*(G4F - c1*r4) ) / sX
All finals run in the transposed [d, i] orientation (beta and bk/bq become
per-partition scalars); the host gather un-transposes.
"""

import math
import sys

import numpy as np

for _p in ("/opt/trn_rl_repo",):
    if _p not in sys.path:
        sys.path.insert(0, _p)

B, N, D, PED = 4, 512, 32, 16
NCORES = 8
RPC = N // 2            # i-rows per core
NJT = N // 128          # j tiles of 128
GW = 128                # stationary columns [h | Fh | F | 1 | pad]
KSH = 10.0
ISD = 1.0 / math.sqrt(D)

# aux1 column layout
A_EJ = 0                # [128, 4]   E_j per j-tile
A_BB = 4                # [128, 32]  beta broadcast
A_BK = 36               # [0:32, 1]  bk
A_BQ = 37               # [0:32, 1]  bq
A_B32 = 38              # [0:32, 1]  beta column
A_EIR = 40              # [0:1, 256] E_i row
A_PR = 296              # [128, 32]  +-1 pair-reduce stationary
A_W = 328

_CACHE = {}


def _patch_act_tables():
    """Make natural_log_exp_and_others the only ACT table set containing our
    functions (exp/ln/identity/copy) so bacc emits exactly one
    ACT_TABLE_LOAD.  Dict length/order is preserved — the set INDEX is the
    runtime act_func_set_id, so entries must not be removed."""
    import concourse.bacc as bacc_mod
    if getattr(bacc_mod, "_act_tables_patched", False):
        return
    orig = bacc_mod.get_activation_tables

    def filtered(arch):
        t = orig(arch)
        target = t.get("natural_log_exp_and_others")
        if not target:
            return t
        return {k: (v if k == "natural_log_exp_and_others" else (v - target))
                for k, v in t.items()}

    bacc_mod.get_activation_tables = filtered
    bacc_mod._act_tables_patched = True


def _build_program():
    import concourse.bacc as bacc
    import concourse.tile as tile
    from concourse import mybir
    from contextlib import ExitStack

    _patch_act_tables()

    fp32 = mybir.dt.float32
    f32r = mybir.dt.float32r
    AF = mybir.ActivationFunctionType
    ADD, MUL = mybir.AluOpType.add, mybir.AluOpType.mult
    SUB = mybir.AluOpType.subtract

    nc = bacc.Bacc("TRN2", target_bir_lowering=False, debug=False,
                   num_devices=NCORES)

    def din(name, shape):
        return nc.dram_tensor(name, shape, fp32, kind="ExternalInput").ap()

    AT = din("AT", [128, NJT * RPC])      # host-permuted [p, (t i)]
    hj = din("hj", [128, NJT * D])        # host-permuted [p, (t d)]
    pewkq = din("pewkq", [PED, N + RPC + 2 * D])   # [peT | peiT | Wk | Wq]
    aux1 = din("aux1", [128, A_W])
    aux2 = din("aux2", [D, 2 * RPC])      # [h_i^T | E_i broadcast]
    out = nc.dram_tensor("out", [D, RPC], fp32, kind="ExternalOutput").ap()

    with tile.TileContext(nc) as tc, ExitStack() as ctx:
        cst = ctx.enter_context(tc.tile_pool(name="cst", bufs=1))
        sb = ctx.enter_context(tc.tile_pool(name="sb", bufs=1))
        fin = ctx.enter_context(tc.tile_pool(name="fin", bufs=1))
        pq = ctx.enter_context(tc.tile_pool(name="pq", bufs=1, space="PSUM"))
        sps = ctx.enter_context(tc.tile_pool(name="sps", bufs=1, space="PSUM"))
        fps = ctx.enter_context(tc.tile_pool(name="fps", bufs=1, space="PSUM"))

        # --- input DMAs: all issued from the idle sync engine (a
        # DMA_DIRECT2D blocks its issuing engine ~650ns), ordered by need ---
        pewkq_sb = cst.tile([PED, N + RPC + 2 * D], fp32, tag="pewkq_sb")
        nc.sync.dma_start(pewkq_sb[:], pewkq[:])
        aux_sb = cst.tile([128, A_W], fp32, tag="aux_sb")
        nc.sync.dma_start(aux_sb[:], aux1[:])
        at_all = cst.tile([128, NJT * RPC], fp32, tag="at_all")
        HW0 = NJT * RPC // 2
        nc.sync.dma_start(at_all[:, 0:HW0], AT[:, 0:HW0])
        hj_sb = cst.tile([128, NJT * D], fp32, tag="hj_sb")
        nc.sync.dma_start(hj_sb[:], hj[:])
        nc.sync.dma_start(at_all[:, HW0:2 * HW0], AT[:, HW0:2 * HW0])
        aux2_sb = cst.tile([D, 2 * RPC], fp32, tag="aux2_sb")
        nc.sync.dma_start(aux2_sb[:], aux2[:])

        ej = aux_sb[:, A_EJ:A_EJ + NJT]
        betab = aux_sb[:, A_BB:A_BB + D]
        bk = aux_sb[0:D, A_BK:A_BK + 1]
        bq = aux_sb[0:D, A_BQ:A_BQ + 1]
        b32 = aux_sb[0:D, A_B32:A_B32 + 1]
        eirow = aux_sb[0:1, A_EIR:A_EIR + RPC]
        hiT = aux2_sb[:, 0:RPC]
        eibt = aux2_sb[:, RPC:2 * RPC]

        # ---------------- consts ------------------------------------------
        zero1 = cst.tile([128, 1], fp32, tag="zero1")
        nc.vector.memset(zero1[:], 0.0)
        eps1 = cst.tile([128, 1], fp32, tag="eps1")
        nc.vector.memset(eps1[:], 1e-8)
        ones128 = cst.tile([1, 128], f32r, tag="ones128")
        nc.vector.memset(ones128.bitcast(fp32)[:], 1.0)
        # dummy first ACT op: hoists the one ACT_TABLE_LOAD off the
        # critical path (it otherwise waits for the first real input)
        warm = cst.tile([128, 1], fp32, tag="warm")
        nc.scalar.activation(warm[:], zero1[:], AF.Exp, bias=zero1[:])

        # ------- sign split rd = 1/(1 + exp(10Ej)*exp(-10Ei)) --------------
        acol = cst.tile([128, NJT], fp32, tag="acol")
        nc.scalar.activation(acol[:], ej, AF.Exp, bias=zero1[:], scale=KSH)
        brow = cst.tile([1, RPC], f32r, tag="brow")
        bbps = pq.tile([128, 2 * RPC], fp32, tag="bbps")
        d1 = sb.tile([128, NJT * RPC], fp32, tag="d1")
        rd = sb.tile([128, NJT * RPC], fp32, tag="rd")

        def mk_rd(jt):
            nc.vector.tensor_scalar(d1[:, jt * RPC:(jt + 1) * RPC],
                                    bbps[:, 0:RPC],
                                    acol[:, jt:jt + 1], 1.0,
                                    op0=MUL, op1=ADD)
            nc.vector.reciprocal_approx_fast(rd[:, jt * RPC:(jt + 1) * RPC],
                                             d1[:, jt * RPC:(jt + 1) * RPC])
        mk_rd(0)

        # ---------------- K^T, Q^T ----------------------------------------
        pewkq_r = cst.tile([PED, N + RPC + 2 * D], f32r, tag="pewkq_r")
        nc.vector.tensor_copy(pewkq_r[:, N:N + RPC + 2 * D],
                              pewkq_sb[:, N:N + RPC + 2 * D])
        nc.vector.tensor_copy(pewkq_r[:, 0:N], pewkq_sb[:, 0:N])
        peT = pewkq_r[:, 0:N]
        peiT = pewkq_r[:, N:N + RPC]
        wk = pewkq_r[:, N + RPC:N + RPC + D]
        wq = pewkq_r[:, N + RPC + D:N + RPC + 2 * D]
        kps = pq.tile([D, 2 * RPC], fp32, tag="kps")   # bank; use 0:RPC
        nc.tensor.matmul(kps[:, 0:RPC], wk, peiT, start=True, stop=True)
        kT = cst.tile([D, RPC], f32r, tag="kT")
        # (K + bk) * (1/sqrt(D))
        nc.vector.tensor_scalar(kT[:], kps[:, 0:RPC], bk, ISD,
                                op0=ADD, op1=MUL)
        qps = pq.tile([D, N], fp32, tag="qps")
        nc.tensor.matmul(qps[:], wq, peT, start=True, stop=True)
        qT = cst.tile([D, N], f32r, tag="qT")
        nc.scalar.activation(qT[:], qps[:], AF.Identity, bias=bq, scale=1.0)
        nc.scalar.activation(brow[:], eirow, AF.Exp, bias=zero1[0:1, :],
                             scale=-KSH)
        nc.tensor.matmul(bbps[:, 0:RPC], ones128[:], brow[:],
                         start=True, stop=True)

        # ------------- rhs columns [h | Fh | F | 1 | pad] ------------------
        rhs_all = cst.tile([128, NJT * GW], f32r, tag="rhs_all")
        rv = rhs_all.rearrange("p (t c) -> p t c", c=GW)
        hv = hj_sb.rearrange("p (t d) -> p t d", d=D)
        Lt = sb.tile([128, NJT * D], fp32, tag="Lt")
        nc.scalar.activation(Lt[:], hj_sb[:], AF.Ln, bias=eps1[:])
        Lv = Lt.rearrange("p (t d) -> p t d", d=D)
        bbv = betab.rearrange("p (t d) -> p t d", t=1).to_broadcast(
            (128, NJT, D))
        Ft = sb.tile([128, NJT * D], fp32, tag="Ft")
        Fv = Ft.rearrange("p (t d) -> p t d", d=D)
        nc.gpsimd.tensor_mul(Fv[:], Lv[:], bbv)        # beta*ln(h)
        ejb = ej.rearrange("p (t o) -> p t o", o=1).to_broadcast((128, NJT, D))
        nc.gpsimd.tensor_tensor(rv[:, :, 2 * D:3 * D], Fv[:], ejb, op=ADD)
        nc.scalar.activation(rv[:, :, 0:D], hv[:], AF.Identity, bias=zero1[:])
        nc.gpsimd.tensor_mul(rv[:, :, D:2 * D], rv[:, :, 2 * D:3 * D], hv[:])
        # ones replicated on cols 96:128 -> P[96:128] = [sX | r4] broadcast
        nc.vector.memset(rv[:, :, 3 * D:GW].bitcast(fp32), 1.0)

        # ---------------- per-j-tile pipeline -----------------------------
        # sallA holds jt0|jt2, sallB jt1|jt3 so V-reads and PE-writes of
        # consecutive tiles land in different PSUM banks.
        sallA = sps.tile([128, 2 * RPC], fp32, tag="sallA")
        sallB = sps.tile([128, 2 * RPC], fp32, tag="sallB")
        P = fps.tile([GW, 2 * RPC], fp32, tag="P")
        XM = cst.tile([128, NJT * 2 * RPC], f32r, tag="XM")
        msk = sb.tile([128, NJT * RPC], fp32, tag="msk")
        for jt in range(NJT):
            bank = (sallA, sallB)[jt % 2]
            sl = slice((jt // 2) * RPC, (jt // 2 + 1) * RPC)
            nc.tensor.matmul(bank[:, sl], qT[:, jt * 128:(jt + 1) * 128],
                             kT[:], start=True, stop=True)
            nc.vector.tensor_mul(msk[:, jt * RPC:(jt + 1) * RPC],
                                 at_all[:, jt * RPC:(jt + 1) * RPC],
                                 bank[:, sl])
            if jt < NJT - 1:
                mk_rd(jt + 1)
            x0 = jt * 2 * RPC
            nc.scalar.activation(XM[:, x0:x0 + RPC],
                                 msk[:, jt * RPC:(jt + 1) * RPC],
                                 AF.Exp, bias=zero1[:])
            m4eng = nc.vector if jt == 0 else nc.gpsimd
            m4eng.tensor_mul(XM[:, x0 + RPC:x0 + 2 * RPC],
                             XM[:, x0:x0 + RPC],
                             rd[:, jt * RPC:(jt + 1) * RPC])
            nc.tensor.matmul(P[:], rhs_all[:, jt * GW:(jt + 1) * GW],
                             XM[:, x0:x0 + 2 * RPC],
                             start=(jt == 0), stop=(jt == NJT - 1))

        # ---------------- finals prep (transposed [d, i]) -----------------
        LiT = fin.tile([D, RPC], fp32, tag="LiT")
        nc.scalar.activation(LiT[:], hiT, AF.Ln, bias=eps1[0:D, :])
        cb = fin.tile([D, RPC], fp32, tag="cb")
        nc.vector.tensor_scalar_mul(cb[:], LiT[:], b32)
        c1 = fin.tile([D, RPC], fp32, tag="c1")
        nc.vector.tensor_add(c1[:], cb[:], eibt)
        # hh = [hiT; hiT*c1] feeds the fused [t3a; t3b] product
        hh = fin.tile([2 * D, RPC], fp32, tag="hh")
        nc.vector.tensor_copy(hh[0:D, :], hiT)
        nc.vector.tensor_mul(hh[D:2 * D, :], hiT, c1[:])
        pairs_r = cst.tile([128, D], f32r, tag="pairs_r")
        nc.vector.tensor_copy(pairs_r[:], aux_sb[:, A_PR:A_PR + D])

        # ---------------- finals ------------------------------------------
        # P rows: 0:32 GXh|G4h, 32:64 GXFh|G4Fh, 64:96 .|G4F,
        #         96:128 [sX | r4] already broadcast (replicated ones cols)
        # numt = g3Fh - c1*g3h + hiT*G4F - hiT*c1*r4 is reduced by ONE
        # pair-sum matmul over M = [t4 | g3Fh | t3a | t3b] (one 32-row
        # group each) with the +-1 stationary pairs_r.
        gx = fin.tile([2 * D, RPC], fp32, tag="gx")
        nc.scalar.activation(gx[:], P[0:2 * D, 0:RPC], AF.Identity,
                             bias=zero1[0:2 * D, :], scale=1.0)
        # reciprocal_approx_fast mis-reads partition-base offsets on HW —
        # keep a dedicated base-0 SBUF copy of the sX rows
        sxs = fin.tile([D, RPC], fp32, tag="sxs")
        nc.scalar.activation(sxs[:], P[3 * D:4 * D, 0:RPC], AF.Identity,
                             bias=zero1[0:D, :], scale=1.0)
        gh = fin.tile([D, RPC], fp32, tag="gh")       # g3h
        nc.vector.tensor_tensor(gh[:], gx[0:D, :], P[0:D, RPC:2 * RPC],
                                op=SUB)
        M = fin.tile([128, RPC], f32r, tag="M")
        nc.vector.tensor_tensor(M[D:2 * D, :], gx[D:2 * D, :],
                                P[D:2 * D, RPC:2 * RPC], op=SUB)  # g3Fh
        nc.vector.tensor_mul(M[2 * D:4 * D, :], hh[:],
                             P[2 * D:4 * D, RPC:2 * RPC])  # [t3a; t3b]
        nc.gpsimd.tensor_mul(M[0:D, :], c1[:], gh[:])          # t4
        nps = pq.tile([D, 2 * RPC], fp32, tag="nps")
        nc.tensor.matmul(nps[:, 0:RPC], pairs_r[:], M[:],
                         start=True, stop=True)
        invsb = fin.tile([D, RPC], fp32, tag="invsb")
        nc.vector.reciprocal_approx_fast(invsb[:], sxs[:])
        res = fin.tile([D, RPC], fp32, tag="res")
        nc.vector.tensor_mul(res[:], nps[:, 0:RPC], invsb[:])
        nc.sync.dma_start(out[:], res[:])

    nc.compile()
    return nc


def _get_program():
    if "nc" not in _CACHE:
        _CACHE["nc"] = _build_program()
    return _CACHE["nc"]


def make_in_maps(h, pe, E, A, Wk, bk, Wq, bq, beta):
    f = lambda x: np.ascontiguousarray(np.asarray(x, dtype=np.float32))
    h, pe, E, A = f(h), f(pe), f(E), f(A)
    Wk, bk, Wq, bq, beta = f(Wk), f(bk), f(Wq), f(bq), f(beta)
    in_maps = []
    for c in range(NCORES):
        b, r = c // 2, c % 2
        isl = slice(r * RPC, (r + 1) * RPC)
        atp = A[isl].T.reshape(NJT, 128, RPC).transpose(1, 0, 2)
        hjp = h[b].reshape(NJT, 128, D).transpose(1, 0, 2)
        pewkq = np.concatenate(
            [pe[b].T, pe[b, isl].T, Wk, Wq], axis=1)
        aux1 = np.zeros((128, A_W), np.float32)
        aux1[:, A_EJ:A_EJ + NJT] = E.reshape(NJT, 128).T
        aux1[:, A_BB:A_BB + D] = beta
        aux1[0:D, A_BK] = bk
        aux1[0:D, A_BQ] = bq
        aux1[0:D, A_B32] = beta
        aux1[0, A_EIR:A_EIR + RPC] = E[isl]
        pr = np.zeros((128, D), np.float32)
        idx = np.arange(D)
        pr[idx, idx] = -1.0          # -t4
        pr[D + idx, idx] = 1.0       # +g3Fh
        pr[2 * D + idx, idx] = 1.0   # +t3a
        pr[3 * D + idx, idx] = -1.0  # -t3b
        aux1[:, A_PR:A_PR + D] = pr
        aux2 = np.empty((D, 2 * RPC), np.float32)
        aux2[:, 0:RPC] = h[b, isl].T
        aux2[:, RPC:2 * RPC] = E[isl]
        in_maps.append({
            "AT": f(atp.reshape(128, NJT * RPC)),
            "hj": f(hjp.reshape(128, NJT * D)),
            "pewkq": f(pewkq),
            "aux1": aux1,
            "aux2": aux2,
        })
    return in_maps


def gather(results):
    out = np.empty((B, N, D), np.float32)
    for c in range(NCORES):
        b, r = c // 2, c % 2
        out[b, r * RPC:(r + 1) * RPC] = results[c]["out"].T
    return out


def _axon_reset():
    try:
        import ctypes
        import jax
        lib = ctypes.CDLL("/opt/axon/libaxon_pjrt.so")
        lib.axon_reset.restype = ctypes.c_int64
        jax.devices()
        lib.axon_reset()
    except Exception:
        pass


def kernel(t=None, h=None, pe=None, E=None, A=None, Wk=None, bk=None,
           Wq=None, bq=None, beta=None, **_unused):
    from concourse.bass_utils import run_bass_kernel_spmd
    nc = _get_program()
    in_maps = make_in_maps(h, pe, E, A, Wk, bk, Wq, bq, beta)
    try:
        res = run_bass_kernel_spmd(nc, in_maps, list(range(NCORES)))
    except Exception:
        # a previously wedged NeuronCore shows up as an opaque runtime
        # error on the first execute — reset the device once and retry
        _axon_reset()
        import time as _time
        _time.sleep(2)
        res = run_bass_kernel_spmd(nc, in_maps, list(range(NCORES)))
    return gather(res.results)


# revision 27
# speedup vs baseline: 1.0591x; 1.0015x over previous
"""Fused graph Fokker-Planck ODE function kernel for Trainium2 (8 NeuronCores).

Sharding: data-parallel over batch B=4 x row-halves (i in [0,256) / [256,512))
-> 8 shards.  Each core computes dh_dt for one (batch, i-half) pair.

Math (per core; S/X/M kept transposed as [j, i] on chip so the j-contraction
matmul needs no transposes):
    S      = A^T * (Q K^T) / sqrt(D)          [j, i]  (elementwise mask)
    X      = exp(S)                            (unnormalized softmax; masked
                                                scores are O(5) so no max sub)
    rd     = 1/(1+exp(10(E_j-E_i))) = 1 - sigmoid(10(E_j-E_i))
             built separably: exp(10Ej)[j] (x) exp(-10Ei)[i] via a PE rank-1
    M4     = X * rd
    F_jd   = E_j + beta_d * ln(h_jd + 1e-8)   (fuses the E and beta*log terms)
    P      = [h | Fh | F | 1]^T @ [X | M4]    one accumulating matmul chain:
             stationary = value columns (padded to 128), moving = [X | M4]
             -> P[0:32]  = GXh | G4h      P[32:64] = GXFh | G4Fh
                P[64:96] = (GXF) | G4F    P[96]    = sX | r4
    c1     = E_i + beta_d * ln(h_id + 1e-8)   [d, i]
    dh^T   = ( (GXFh-G4Fh) - c1*(GXh-G4h) + h_i^T*(G4F - c1*r4) ) / sX
All finals run in the transposed [d, i] orientation (beta and bk/bq become
per-partition scalars); the host gather un-transposes.
"""

import math
import sys

import numpy as np

for _p in ("/opt/trn_rl_repo",):
    if _p not in sys.path:
        sys.path.insert(0, _p)

B, N, D, PED = 4, 512, 32, 16
NCORES = 8
RPC = N // 2            # i-rows per core
NJT = N // 128          # j tiles of 128
GW = 128                # stationary columns [h | Fh | F | 1 | pad]
KSH = 10.0
ISD = 1.0 / math.sqrt(D)

# aux1 column layout
A_EJ = 0                # [128, 4]   E_j per j-tile
A_BB = 4                # [128, 32]  beta broadcast
A_BK = 36               # [0:32, 1]  bk
A_BQ = 37               # [0:32, 1]  bq
A_B32 = 38              # [0:32, 1]  beta column
A_EIR = 40              # [0:1, 256] E_i row
A_PR = 296              # [128, 32]  +-1 pair-reduce stationary
A_W = 328

_CACHE = {}


def _patch_act_tables():
    """Make natural_log_exp_and_others the only ACT table set containing our
    functions (exp/ln/identity/copy) so bacc emits exactly one
    ACT_TABLE_LOAD.  Dict length/order is preserved — the set INDEX is the
    runtime act_func_set_id, so entries must not be removed."""
    import concourse.bacc as bacc_mod
    if getattr(bacc_mod, "_act_tables_patched", False):
        return
    orig = bacc_mod.get_activation_tables

    def filtered(arch):
        t = orig(arch)
        target = t.get("natural_log_exp_and_others")
        if not target:
            return t
        return {k: (v if k == "natural_log_exp_and_others" else (v - target))
                for k, v in t.items()}

    bacc_mod.get_activation_tables = filtered
    bacc_mod._act_tables_patched = True


def _build_program():
    import concourse.bacc as bacc
    import concourse.tile as tile
    from concourse import mybir
    from contextlib import ExitStack

    _patch_act_tables()

    fp32 = mybir.dt.float32
    f32r = mybir.dt.float32r
    AF = mybir.ActivationFunctionType
    ADD, MUL = mybir.AluOpType.add, mybir.AluOpType.mult
    SUB = mybir.AluOpType.subtract

    nc = bacc.Bacc("TRN2", target_bir_lowering=False, debug=False,
                   num_devices=NCORES)

    def din(name, shape):
        return nc.dram_tensor(name, shape, fp32, kind="ExternalInput").ap()

    AT = din("AT", [128, NJT * RPC])      # host-permuted [p, (t i)]
    hj = din("hj", [128, NJT * D])        # host-permuted [p, (t d)]
    pewkq = din("pewkq", [PED, N + RPC + 2 * D])   # [peT | peiT | Wk | Wq]
    aux1 = din("aux1", [128, A_W])
    aux2 = din("aux2", [D, 2 * RPC])      # [h_i^T | E_i broadcast]
    out = nc.dram_tensor("out", [D, RPC], fp32, kind="ExternalOutput").ap()

    with tile.TileContext(nc) as tc, ExitStack() as ctx:
        cst = ctx.enter_context(tc.tile_pool(name="cst", bufs=1))
        sb = ctx.enter_context(tc.tile_pool(name="sb", bufs=1))
        fin = ctx.enter_context(tc.tile_pool(name="fin", bufs=1))
        pq = ctx.enter_context(tc.tile_pool(name="pq", bufs=1, space="PSUM"))
        sps = ctx.enter_context(tc.tile_pool(name="sps", bufs=1, space="PSUM"))
        fps = ctx.enter_context(tc.tile_pool(name="fps", bufs=1, space="PSUM"))

        # --- input DMAs: all issued from the idle sync engine (a
        # DMA_DIRECT2D blocks its issuing engine ~650ns), ordered by need ---
        pewkq_sb = cst.tile([PED, N + RPC + 2 * D], fp32, tag="pewkq_sb")
        nc.sync.dma_start(pewkq_sb[:], pewkq[:])
        aux_sb = cst.tile([128, A_W], fp32, tag="aux_sb")
        nc.sync.dma_start(aux_sb[:], aux1[:])
        at_all = cst.tile([128, NJT * RPC], fp32, tag="at_all")
        HW0 = NJT * RPC // 2
        nc.sync.dma_start(at_all[:, 0:HW0], AT[:, 0:HW0])
        hj_sb = cst.tile([128, NJT * D], fp32, tag="hj_sb")
        nc.sync.dma_start(hj_sb[:], hj[:])
        nc.sync.dma_start(at_all[:, HW0:2 * HW0], AT[:, HW0:2 * HW0])
        aux2_sb = cst.tile([D, 2 * RPC], fp32, tag="aux2_sb")
        nc.sync.dma_start(aux2_sb[:], aux2[:])

        ej = aux_sb[:, A_EJ:A_EJ + NJT]
        betab = aux_sb[:, A_BB:A_BB + D]
        bk = aux_sb[0:D, A_BK:A_BK + 1]
        bq = aux_sb[0:D, A_BQ:A_BQ + 1]
        b32 = aux_sb[0:D, A_B32:A_B32 + 1]
        eirow = aux_sb[0:1, A_EIR:A_EIR + RPC]
        hiT = aux2_sb[:, 0:RPC]
        eibt = aux2_sb[:, RPC:2 * RPC]

        # ---------------- consts ------------------------------------------
        zero1 = cst.tile([128, 1], fp32, tag="zero1")
        nc.vector.memset(zero1[:], 0.0)
        eps1 = cst.tile([128, 1], fp32, tag="eps1")
        nc.vector.memset(eps1[:], 1e-8)
        ones128 = cst.tile([1, 128], f32r, tag="ones128")
        nc.vector.memset(ones128.bitcast(fp32)[:], 1.0)
        # dummy first ACT op: hoists the one ACT_TABLE_LOAD off the
        # critical path (it otherwise waits for the first real input)
        warm = cst.tile([128, 1], fp32, tag="warm")
        nc.scalar.activation(warm[:], zero1[:], AF.Exp, bias=zero1[:])

        # ------- sign split rd = 1/(1 + exp(10Ej)*exp(-10Ei)) --------------
        acol = cst.tile([128, NJT], fp32, tag="acol")
        nc.scalar.activation(acol[:], ej, AF.Exp, bias=zero1[:], scale=KSH)
        brow = cst.tile([1, RPC], f32r, tag="brow")
        bbps = pq.tile([128, 2 * RPC], fp32, tag="bbps")
        d1 = sb.tile([128, NJT * RPC], fp32, tag="d1")
        rd = sb.tile([128, NJT * RPC], fp32, tag="rd")

        def mk_rd(jt):
            nc.vector.tensor_scalar(d1[:, jt * RPC:(jt + 1) * RPC],
                                    bbps[:, 0:RPC],
                                    acol[:, jt:jt + 1], 1.0,
                                    op0=MUL, op1=ADD)
            nc.vector.reciprocal_approx_fast(rd[:, jt * RPC:(jt + 1) * RPC],
                                             d1[:, jt * RPC:(jt + 1) * RPC])
        mk_rd(0)

        # ---------------- K^T, Q^T ----------------------------------------
        pewkq_r = cst.tile([PED, N + RPC + 2 * D], f32r, tag="pewkq_r")
        nc.vector.tensor_copy(pewkq_r[:, N:N + RPC + 2 * D],
                              pewkq_sb[:, N:N + RPC + 2 * D])
        nc.vector.tensor_copy(pewkq_r[:, 0:N], pewkq_sb[:, 0:N])
        peT = pewkq_r[:, 0:N]
        peiT = pewkq_r[:, N:N + RPC]
        wk = pewkq_r[:, N + RPC:N + RPC + D]
        wq = pewkq_r[:, N + RPC + D:N + RPC + 2 * D]
        kps = pq.tile([D, 2 * RPC], fp32, tag="kps")   # bank; use 0:RPC
        nc.tensor.matmul(kps[:, 0:RPC], wk, peiT, start=True, stop=True)
        kT = cst.tile([D, RPC], f32r, tag="kT")
        # (K + bk) * (1/sqrt(D))
        nc.vector.tensor_scalar(kT[:], kps[:, 0:RPC], bk, ISD,
                                op0=ADD, op1=MUL)
        qps = pq.tile([D, N], fp32, tag="qps")
        nc.tensor.matmul(qps[:], wq, peT, start=True, stop=True)
        qT = cst.tile([D, N], f32r, tag="qT")
        nc.scalar.activation(qT[:], qps[:], AF.Identity, bias=bq, scale=1.0)
        nc.scalar.activation(brow[:], eirow, AF.Exp, bias=zero1[0:1, :],
                             scale=-KSH)
        nc.tensor.matmul(bbps[:, 0:RPC], ones128[:], brow[:],
                         start=True, stop=True)

        # ------------- rhs columns [h | Fh | F | 1 | pad] ------------------
        rhs_all = cst.tile([128, NJT * GW], f32r, tag="rhs_all")
        rv = rhs_all.rearrange("p (t c) -> p t c", c=GW)
        hv = hj_sb.rearrange("p (t d) -> p t d", d=D)
        Lt = sb.tile([128, NJT * D], fp32, tag="Lt")
        nc.scalar.activation(Lt[:], hj_sb[:], AF.Ln, bias=eps1[:])
        Lv = Lt.rearrange("p (t d) -> p t d", d=D)
        bbv = betab.rearrange("p (t d) -> p t d", t=1).to_broadcast(
            (128, NJT, D))
        Ft = sb.tile([128, NJT * D], fp32, tag="Ft")
        Fv = Ft.rearrange("p (t d) -> p t d", d=D)
        nc.gpsimd.tensor_mul(Fv[:], Lv[:], bbv)        # beta*ln(h)
        ejb = ej.rearrange("p (t o) -> p t o", o=1).to_broadcast((128, NJT, D))
        nc.gpsimd.tensor_tensor(rv[:, :, 2 * D:3 * D], Fv[:], ejb, op=ADD)
        nc.scalar.activation(rv[:, :, 0:D], hv[:], AF.Identity, bias=zero1[:])
        nc.gpsimd.tensor_mul(rv[:, :, D:2 * D], rv[:, :, 2 * D:3 * D], hv[:])
        # ones replicated on cols 96:128 -> P[96:128] = [sX | r4] broadcast
        nc.vector.memset(rv[:, :, 3 * D:GW].bitcast(fp32), 1.0)

        # ---------------- per-j-tile pipeline -----------------------------
        # sallA holds jt0|jt2, sallB jt1|jt3 so V-reads and PE-writes of
        # consecutive tiles land in different PSUM banks.
        sallA = sps.tile([128, 2 * RPC], fp32, tag="sallA")
        sallB = sps.tile([128, 2 * RPC], fp32, tag="sallB")
        P = fps.tile([GW, 2 * RPC], fp32, tag="P")
        XM = cst.tile([128, NJT * 2 * RPC], f32r, tag="XM")
        msk = sb.tile([128, NJT * RPC], fp32, tag="msk")
        for jt in range(NJT):
            bank = (sallA, sallB)[jt % 2]
            sl = slice((jt // 2) * RPC, (jt // 2 + 1) * RPC)
            nc.tensor.matmul(bank[:, sl], qT[:, jt * 128:(jt + 1) * 128],
                             kT[:], start=True, stop=True)
            nc.vector.tensor_mul(msk[:, jt * RPC:(jt + 1) * RPC],
                                 at_all[:, jt * RPC:(jt + 1) * RPC],
                                 bank[:, sl])
            if jt < NJT - 1:
                mk_rd(jt + 1)
            x0 = jt * 2 * RPC
            nc.scalar.activation(XM[:, x0:x0 + RPC],
                                 msk[:, jt * RPC:(jt + 1) * RPC],
                                 AF.Exp, bias=zero1[:])
            m4eng = nc.vector if jt in (0, 3) else nc.gpsimd
            m4eng.tensor_mul(XM[:, x0 + RPC:x0 + 2 * RPC],
                             XM[:, x0:x0 + RPC],
                             rd[:, jt * RPC:(jt + 1) * RPC])
            nc.tensor.matmul(P[:], rhs_all[:, jt * GW:(jt + 1) * GW],
                             XM[:, x0:x0 + 2 * RPC],
                             start=(jt == 0), stop=(jt == NJT - 1))

        # ---------------- finals prep (transposed [d, i]) -----------------
        LiT = fin.tile([D, RPC], fp32, tag="LiT")
        nc.scalar.activation(LiT[:], hiT, AF.Ln, bias=eps1[0:D, :])
        cb = fin.tile([D, RPC], fp32, tag="cb")
        nc.vector.tensor_scalar_mul(cb[:], LiT[:], b32)
        c1 = fin.tile([D, RPC], fp32, tag="c1")
        nc.vector.tensor_add(c1[:], cb[:], eibt)
        # hh = [hiT; hiT*c1] feeds the fused [t3a; t3b] product
        hh = fin.tile([2 * D, RPC], fp32, tag="hh")
        nc.vector.tensor_copy(hh[0:D, :], hiT)
        nc.vector.tensor_mul(hh[D:2 * D, :], hiT, c1[:])
        pairs_r = cst.tile([128, D], f32r, tag="pairs_r")
        nc.vector.tensor_copy(pairs_r[:], aux_sb[:, A_PR:A_PR + D])

        # ---------------- finals ------------------------------------------
        # P rows: 0:32 GXh|G4h, 32:64 GXFh|G4Fh, 64:96 .|G4F,
        #         96:128 [sX | r4] already broadcast (replicated ones cols)
        # numt = g3Fh - c1*g3h + hiT*G4F - hiT*c1*r4 is reduced by ONE
        # pair-sum matmul over M = [t4 | g3Fh | t3a | t3b] (one 32-row
        # group each) with the +-1 stationary pairs_r.
        gx = fin.tile([2 * D, RPC], fp32, tag="gx")
        nc.scalar.activation(gx[:], P[0:2 * D, 0:RPC], AF.Identity,
                             bias=zero1[0:2 * D, :], scale=1.0)
        # reciprocal_approx_fast mis-reads partition-base offsets on HW —
        # keep a dedicated base-0 SBUF copy of the sX rows
        sxs = fin.tile([D, RPC], fp32, tag="sxs")
        nc.scalar.activation(sxs[:], P[3 * D:4 * D, 0:RPC], AF.Identity,
                             bias=zero1[0:D, :], scale=1.0)
        gh = fin.tile([D, RPC], fp32, tag="gh")       # g3h
        nc.vector.tensor_tensor(gh[:], gx[0:D, :], P[0:D, RPC:2 * RPC],
                                op=SUB)
        M = fin.tile([128, RPC], f32r, tag="M")
        nc.vector.tensor_tensor(M[D:2 * D, :], gx[D:2 * D, :],
                                P[D:2 * D, RPC:2 * RPC], op=SUB)  # g3Fh
        nc.vector.tensor_mul(M[2 * D:4 * D, :], hh[:],
                             P[2 * D:4 * D, RPC:2 * RPC])  # [t3a; t3b]
        nc.gpsimd.tensor_mul(M[0:D, :], c1[:], gh[:])          # t4
        nps = pq.tile([D, 2 * RPC], fp32, tag="nps")
        nc.tensor.matmul(nps[:, 0:RPC], pairs_r[:], M[:],
                         start=True, stop=True)
        invsb = fin.tile([D, RPC], fp32, tag="invsb")
        nc.vector.reciprocal_approx_fast(invsb[:], sxs[:])
        res = fin.tile([D, RPC], fp32, tag="res")
        nc.vector.tensor_mul(res[:], nps[:, 0:RPC], invsb[:])
        nc.sync.dma_start(out[:], res[:])

    nc.compile()
    return nc


def _get_program():
    if "nc" not in _CACHE:
        _CACHE["nc"] = _build_program()
    return _CACHE["nc"]


def make_in_maps(h, pe, E, A, Wk, bk, Wq, bq, beta):
    f = lambda x: np.ascontiguousarray(np.asarray(x, dtype=np.float32))
    h, pe, E, A = f(h), f(pe), f(E), f(A)
    Wk, bk, Wq, bq, beta = f(Wk), f(bk), f(Wq), f(bq), f(beta)
    in_maps = []
    for c in range(NCORES):
        b, r = c // 2, c % 2
        isl = slice(r * RPC, (r + 1) * RPC)
        atp = A[isl].T.reshape(NJT, 128, RPC).transpose(1, 0, 2)
        hjp = h[b].reshape(NJT, 128, D).transpose(1, 0, 2)
        pewkq = np.concatenate(
            [pe[b].T, pe[b, isl].T, Wk, Wq], axis=1)
        aux1 = np.zeros((128, A_W), np.float32)
        aux1[:, A_EJ:A_EJ + NJT] = E.reshape(NJT, 128).T
        aux1[:, A_BB:A_BB + D] = beta
        aux1[0:D, A_BK] = bk
        aux1[0:D, A_BQ] = bq
        aux1[0:D, A_B32] = beta
        aux1[0, A_EIR:A_EIR + RPC] = E[isl]
        pr = np.zeros((128, D), np.float32)
        idx = np.arange(D)
        pr[idx, idx] = -1.0          # -t4
        pr[D + idx, idx] = 1.0       # +g3Fh
        pr[2 * D + idx, idx] = 1.0   # +t3a
        pr[3 * D + idx, idx] = -1.0  # -t3b
        aux1[:, A_PR:A_PR + D] = pr
        aux2 = np.empty((D, 2 * RPC), np.float32)
        aux2[:, 0:RPC] = h[b, isl].T
        aux2[:, RPC:2 * RPC] = E[isl]
        in_maps.append({
            "AT": f(atp.reshape(128, NJT * RPC)),
            "hj": f(hjp.reshape(128, NJT * D)),
            "pewkq": f(pewkq),
            "aux1": aux1,
            "aux2": aux2,
        })
    return in_maps


def gather(results):
    out = np.empty((B, N, D), np.float32)
    for c in range(NCORES):
        b, r = c // 2, c % 2
        out[b, r * RPC:(r + 1) * RPC] = results[c]["out"].T
    return out


def _axon_reset():
    try:
        import ctypes
        import jax
        lib = ctypes.CDLL("/opt/axon/libaxon_pjrt.so")
        lib.axon_reset.restype = ctypes.c_int64
        jax.devices()
        lib.axon_reset()
    except Exception:
        pass


def kernel(t=None, h=None, pe=None, E=None, A=None, Wk=None, bk=None,
           Wq=None, bq=None, beta=None, **_unused):
    from concourse.bass_utils import run_bass_kernel_spmd
    nc = _get_program()
    in_maps = make_in_maps(h, pe, E, A, Wk, bk, Wq, bq, beta)
    try:
        res = run_bass_kernel_spmd(nc, in_maps, list(range(NCORES)))
    except Exception:
        # a previously wedged NeuronCore shows up as an opaque runtime
        # error on the first execute — reset the device once and retry
        _axon_reset()
        import time as _time
        _time.sleep(2)
        res = run_bass_kernel_spmd(nc, in_maps, list(range(NCORES)))
    return gather(res.results)
